# revision 32
# baseline (speedup 1.0000x reference)
"""Trainium2 Bass kernel: 8-layer decoder-only LM (dense transformer), TP=8.

Sharding (per NeuronCore c of 8):
  - Residual stream x is FEATURE-SHARDED: core c owns feature rows
    [128c, 128c+128) in feature-major layout [128, T] (T = B*S = 2048 tokens).
  - Attention: 2 heads per core (qkv column-sharded in head order).
  - proj / fc2 COLUMN-sharded on the output dim -> deltas come back
    feature-sharded (proj needs AllGather of per-head outputs o; fc2 needs a
    ReduceScatter of partial [D, T] sums).
  - LayerNorm: per-core partial sums over the 128-feature shard via
    ones-matmuls, tiny AllReduce of [2, T] stats, normalize shard, cast fp16,
    AllGather the normalized activations h for the next matmul.
  - LN gains folded into the following weight matrices on the host; LN biases
    are zero in this model (asserted at prep time).
  - Tied LM head: vocab-sharded (4000 rows/core), computed token-major
    (h tiles stationary) so the logits DMA out contiguous.

All matmuls run in fp16 (fp32 PSUM accumulate). Attention scores for this
model stay within +-3 (measured), so exp() fits fp16 comfortably.
The two batch elements are pipelined (emission interleaved per stage) so
collective latency hides under the other batch's compute.
"""

import numpy as np

import concourse.mybir as mybir
import concourse.bacc as bacc
from concourse import tile
from concourse.bass_utils import run_bass_kernel_spmd

dt = mybir.dt
AF = mybir.ActivationFunctionType
OP = mybir.AluOpType

# Model dims (fixed for this problem)
V, D, H, L, B, S = 32000, 1024, 16, 8, 2, 1024
DH = D // H            # 64
INNER = 4 * D          # 4096
EPS = 1e-5
NC = 8                 # cores
MASK_NEG = -30000.0    # additive causal mask (fp16-safe; exp(-30000/8) == 0)


class Cfg:
    def __init__(self, V=V, L=L, S=S, NBLK=512, no_coll=False, skip=()):
        self.no_coll = no_coll
        self.skip = set(skip)          # timing-bisect: {"attn","mlp","head"}
        self.V, self.L, self.S = V, L, S
        self.T = B * S
        self.NBLK = NBLK                 # token block (matmul N) size
        self.DSH = D // NC               # 128   feature shard
        self.HPC = H // NC               # 2     heads per core
        self.F1 = INNER // NC            # 512   fc1 out shard
        self.VS = self.V // NC           # vocab shard
        self.KT = D // 128               # 8     K tiles over D
        self.NB_B = S // NBLK            # N blocks per batch element
        self.TKT_B = S // 128            # tk tiles per batch element
        self.F1T = self.F1 // 128        # 4     fc2 K tiles
        self.DT = D // 128               # 8     output feature tiles


def build_program(cfg: Cfg):
    """Build the SPMD Bass program (identical on all 8 cores)."""
    c = cfg
    nc = bacc.Bacc("TRN2", target_bir_lowering=False, debug=False,
                   enable_asserts=False, num_devices=NC)

    f32, f16 = dt.float32, dt.float16

    # ---- DRAM I/O (per-core contents supplied via in_maps) ----
    x0_d = nc.dram_tensor("x0", [c.DSH, c.T], f32, kind="ExternalInput").ap()
    wqkvT_d = nc.dram_tensor("wqkvT", [c.L, D, 3 * c.DSH], f16, kind="ExternalInput").ap()
    wprojT_d = nc.dram_tensor("wprojT", [c.L, D, c.DSH], f16, kind="ExternalInput").ap()
    wfc1T_d = nc.dram_tensor("wfc1T", [c.L, D, c.F1], f16, kind="ExternalInput").ap()
    wfc2T_d = nc.dram_tensor("wfc2T", [c.L, c.F1, D], f16, kind="ExternalInput").ap()
    embT_d = nc.dram_tensor("embT", [D, c.VS], f16, kind="ExternalInput").ap()
    cw1_d = nc.dram_tensor("cw1", [c.L, c.F1], f16, kind="ExternalInput").ap()
    cemb_d = nc.dram_tensor("cemb", [1, c.VS], f16, kind="ExternalInput").ap()
    cosT_d = nc.dram_tensor("cosT", [c.DSH, c.T], f16, kind="ExternalInput").ap()
    sinT_d = nc.dram_tensor("sinT", [c.DSH, c.T], f16, kind="ExternalInput").ap()
    nmask = c.NBLK // 128
    mask_d = nc.dram_tensor("masks", [128, nmask, c.NBLK], f16, kind="ExternalInput").ap()
    pblk_d = nc.dram_tensor("pblk", [128, 128], f16, kind="ExternalInput").ap()
    out_d = nc.dram_tensor("logits", [c.T, c.VS], f32, kind="ExternalOutput").ap()

    RG = [list(range(NC))]

    def _collective(kind, op, cin, cout):
        if c.no_coll:
            # timing-bisect mode: local DMA with the right shapes (numerics wrong)
            ish, osh = cin.shape, cout.shape
            if ish == osh:
                nc.sync.dma_start(cout[:], cin[:])
            elif ish[0] < osh[0]:      # AllGather stand-in
                for rr_ in range(osh[0] // ish[0]):
                    nc.sync.dma_start(cout[rr_ * ish[0]:(rr_ + 1) * ish[0], :], cin[:])
            else:                      # ReduceScatter stand-in
                nc.sync.dma_start(cout[:], cin[0:osh[0], :])
            return
        nc.gpsimd.collective_compute(kind, op, replica_groups=RG,
                                     ins=[cin.opt()], outs=[cout.opt()])

    def _blocked_load(dst_tile, src2d, kt, split=False):
        # DRAM [kt*128, M] -> SBUF [128, kt*M] (col block k = rows 128k..)
        m = src2d.shape[-1]
        if split:
            for k in range(kt):
                nc.sync.dma_start(dst_tile[:, k * m:(k + 1) * m],
                                  src2d[k * 128:(k + 1) * 128, :])
            return
        nc.sync.dma_start(
            dst_tile[:].rearrange("p (k m) -> p k m", k=kt),
            src2d.rearrange("(k p) m -> p k m", p=128))

    from contextlib import ExitStack
    with tile.TileContext(nc) as tc:
        with ExitStack() as es:
            es.enter_context(nc.allow_low_precision(
                reason="fp16 activations by design; fp32 accumulate in PSUM"))
            consts = es.enter_context(tc.tile_pool(name="consts", bufs=1))
            wqp = es.enter_context(tc.tile_pool(name="wq", bufs=2))
            wpp = es.enter_context(tc.tile_pool(name="wp", bufs=2))
            w1p = es.enter_context(tc.tile_pool(name="w1", bufs=2))
            w2p = es.enter_context(tc.tile_pool(name="w2", bufs=1))
            gathp = es.enter_context(tc.tile_pool(name="gath", bufs=2))
            qkp = es.enter_context(tc.tile_pool(name="qk", bufs=2))
            vtp = es.enter_context(tc.tile_pool(name="vtm", bufs=2 * c.TKT_B * B))
            gactp = es.enter_context(tc.tile_pool(name="gact", bufs=2))
            probp = es.enter_context(tc.tile_pool(name="probs", bufs=3))
            ocp = es.enter_context(tc.tile_pool(name="oc", bufs=2))
            stagep = es.enter_context(tc.tile_pool(name="stage", bufs=2))
            rowsp = es.enter_context(tc.tile_pool(name="rows", bufs=2))
            rows1p = es.enter_context(tc.tile_pool(name="rows1", bufs=1))
            xtp = es.enter_context(tc.tile_pool(name="xt", bufs=2))
            embp = es.enter_context(tc.tile_pool(name="emb", bufs=2))
            psmm = es.enter_context(tc.tile_pool(name="psmm", bufs=2, space="PSUM"))
            pssc = es.enter_context(tc.tile_pool(name="pssc", bufs=2, space="PSUM"))
            psov = es.enter_context(tc.tile_pool(name="pso", bufs=2, space="PSUM"))
            psx = es.enter_context(tc.tile_pool(name="psx", bufs=2, space="PSUM"))
            dram = es.enter_context(tc.tile_pool(name="dram", bufs=2, space="DRAM"))
            # ---------------- persistent constants ----------------
            xres = consts.tile([c.DSH, c.T], f32, tag="xres")
            nc.sync.dma_start(xres[:], x0_d)
            cosT = consts.tile([c.DSH, c.T], f16, tag="cosT")
            sinT = consts.tile([c.DSH, c.T], f16, tag="sinT")
            nc.sync.dma_start(cosT[:], cosT_d)
            nc.sync.dma_start(sinT[:], sinT_d)
            masks = consts.tile([128, nmask * c.NBLK], f16, tag="masks")
            nc.sync.dma_start(masks[:], mask_d.rearrange("p m n -> p (m n)"))
            pblk = consts.tile([128, 128], f16, tag="pblk")
            nc.sync.dma_start(pblk[:], pblk_d)
            ones_col = consts.tile([128, 1], f16, tag="ones_col")
            nc.vector.memset(ones_col[:], 1.0)
            ones_row = consts.tile([1, 128], f16, tag="ones_row")
            nc.vector.memset(ones_row[:], 1.0)

            # ---------------- stage helpers (emit IR; pipelined by caller) ---
            def ln_stats(b, defer=False):
                """Partial LN stats of xres b-half -> AllReduce; returns the
                stats DRAM tile [2, S] (summed over cores) for ln_apply."""
                t0 = b * c.S
                xc = xtp.tile([c.DSH, c.S], f16, tag="xcast")
                xsq = xtp.tile([c.DSH, c.S], f16, tag="xsq")
                # scale by 1/16 so squares stay well inside fp16 range
                nc.scalar.activation(xc[:], xres[:, t0:t0 + c.S], AF.Copy,
                                     scale=1.0 / 16)
                nc.vector.tensor_mul(xsq[:], xc[:], xc[:])
                xcf = None
                if defer:
                    # gather the RAW x/16 shard now, concurrent with stats-AR;
                    # mean/rstd are fixed up inside the consumer instead.
                    agr_in = dram.tile([c.DSH, c.S], f16, tag="agr_in")
                    agr_out = dram.tile([D, c.S], f16, tag="agr_out",
                                        addr_space=("Local" if c.no_coll else "Shared"))
                    nc.sync.dma_start(agr_in[:], xc[:])
                    _collective("AllGather", OP.bypass, agr_in, agr_out)
                    xcf = gathp.tile([128, c.KT * c.S], f16, tag="gfull")
                    _blocked_load(xcf, agr_out[:], c.KT, split=True)
                st_in = dram.tile([2, c.S], f32, tag="st_in")
                st_out = dram.tile([2, c.S], f32, tag="st_out", addr_space=("Local" if c.no_coll else "Shared"))
                for nb in range(c.NB_B):
                    n0 = nb * c.NBLK
                    sps = psx.tile([33, c.NBLK], f32, tag="aux")
                    nc.tensor.matmul(sps[0:1, :], ones_col[:],
                                     xc[:, n0:n0 + c.NBLK], start=True, stop=True)
                    nc.tensor.matmul(sps[32:33, :], ones_col[:],
                                     xsq[:, n0:n0 + c.NBLK], start=True, stop=True)
                    srow = rowsp.tile([33, c.NBLK], f32, tag="srow")
                    nc.vector.tensor_copy(srow[0:1, :], sps[0:1, :])
                    nc.vector.tensor_copy(srow[32:33, :], sps[32:33, :])
                    nc.sync.dma_start(st_in[0:1, n0:n0 + c.NBLK], srow[0:1, :])
                    nc.sync.dma_start(st_in[1:2, n0:n0 + c.NBLK], srow[32:33, :])
                _collective("AllReduce", OP.add, st_in, st_out)
                return st_out, xcf

            def ln_apply(b, st_out, lname, defer=False):
                """Row math + normalize shard + AllGather; returns gathered
                h tile [128, KT*S] fp16 (col block k = feature rows 128k..)."""
                t0 = b * c.S
                nchunk = c.S // 128
                mrow = rowsp.tile([nchunk, 128], f32, tag="mrow")
                vrow = rowsp.tile([nchunk, 128], f32, tag="vrow")
                nc.sync.dma_start(
                    mrow[:], st_out[0:1, :].rearrange("o (p n) -> (o p) n", p=nchunk))
                nc.sync.dma_start(
                    vrow[:], st_out[1:2, :].rearrange("o (p n) -> (o p) n", p=nchunk))
                # mu = sum(x/16)*16/D ; ms = sum((x/16)^2)*256/D
                mu = rowsp.tile([nchunk, 128], f32, tag="mu")
                nc.scalar.activation(mu[:], mrow[:], AF.Copy, scale=16.0 / D)
                musq = rowsp.tile([nchunk, 128], f32, tag="musq")
                nc.vector.tensor_mul(musq[:], mu[:], mu[:])
                # var = vrow*256/D - mu^2  (one scalar_tensor_tensor)
                var = rowsp.tile([nchunk, 128], f32, tag="var")
                nc.vector.scalar_tensor_tensor(
                    var[:], vrow[:], 256.0 / D, musq[:],
                    op0=OP.mult, op1=OP.subtract)
                nc.vector.tensor_scalar_add(var[:], var[:], EPS)
                sd = rowsp.tile([nchunk, 128], f32, tag="sd")
                nc.scalar.sqrt(sd[:], var[:])
                sig = rowsp.tile([nchunk, 128], f32, tag="sig")
                nc.vector.reciprocal(sig[:], sd[:])
                a16 = rowsp.tile([nchunk, 128], f16, tag="a16")
                b16 = rowsp.tile([nchunk, 128], f16, tag="b16")
                a32 = None
                if defer:
                    nc.scalar.activation(a16[:], sig[:], AF.Copy, scale=16.0)
                    nc.scalar.activation(b16[:], mu[:], AF.Copy, scale=-1.0 / 16)
                    a32 = rowsp.tile([nchunk, 128], f32, tag="a32")
                    nc.scalar.activation(a32[:], sig[:], AF.Copy, scale=16.0)
                else:
                    nc.vector.tensor_copy(a16[:], sig[:])
                    nc.vector.tensor_mul(b16[:], mu[:], sig[:])
                # re-partition [S/128,128] -> rows [1, S]
                arow = rows1p.tile([1, c.S], f16, tag="arow")
                brow = rows1p.tile([1, c.S], f16, tag="brow")
                nc.sync.dma_start(
                    arow[:].rearrange("o (p n) -> o p n", p=nchunk), a16[:])
                nc.sync.dma_start(
                    brow[:].rearrange("o (p n) -> o p n", p=nchunk), b16[:])
                if defer:
                    return arow, brow, a32
                # normalize shard -> h fp16
                hc = xtp.tile([c.DSH, c.S], f16, tag="hc")
                for nb in range(c.NB_B):
                    n0 = nb * c.NBLK
                    abps = psx.tile([128, c.NBLK], f32, tag="aux")
                    nc.tensor.matmul(abps[:], ones_row[:], arow[0:1, n0:n0 + c.NBLK],
                                     start=True, stop=True)
                    tmp = xtp.tile([c.DSH, c.NBLK], f32, tag="ropet1")
                    nc.vector.tensor_mul(tmp[:], xres[:, t0 + n0:t0 + n0 + c.NBLK],
                                         abps[:])
                    bbps = psx.tile([128, c.NBLK], f32, tag="aux")
                    nc.tensor.matmul(bbps[:], ones_row[:], brow[0:1, n0:n0 + c.NBLK],
                                     start=True, stop=True)
                    nc.vector.tensor_sub(hc[:, n0:n0 + c.NBLK], tmp[:], bbps[:])
                # AllGather h
                ag_in = dram.tile([c.DSH, c.S], f16, tag=f"agin_{lname}")
                ag_out = dram.tile([D, c.S], f16, tag=f"agout_{lname}", addr_space=("Local" if c.no_coll else "Shared"))
                nc.sync.dma_start(ag_in[:], hc[:])
                _collective("AllGather", OP.bypass, ag_in, ag_out)
                hf = gathp.tile([128, c.KT * c.S], f16, tag="gfull")
                _blocked_load(hf, ag_out[:], c.KT, split=True)
                return hf

            def qkv_stage(b, h1, wq):
                """q,k (rope applied) [128, S] fp16 + v token-major tiles."""
                t0 = b * c.S
                qt = qkp.tile([c.DSH, c.S], f16, tag="q")
                kt = qkp.tile([c.DSH, c.S], f16, tag="k")
                for nb in range(c.NB_B):
                    n0 = nb * c.NBLK
                    pss, raws = [], []
                    for mi in (0, 1):
                        ps = psmm.tile([128, c.NBLK], f32, tag="mm")
                        for k in range(c.KT):
                            nc.tensor.matmul(
                                ps[:],
                                wq[:, k * 3 * c.DSH + mi * 128:
                                   k * 3 * c.DSH + mi * 128 + 128],
                                h1[:, k * c.S + n0:k * c.S + n0 + c.NBLK],
                                start=(k == 0), stop=(k == c.KT - 1))
                        raw = stagep.tile([128, c.NBLK], f16, tag="rraw")
                        nc.scalar.copy(raw[:], ps[:])
                        pss.append(ps)
                        raws.append(raw)
                    # rope: dest = ps*cos + (Pblk@ps)*sin ; swap matmuls for
                    # both tensors come after both psum groups so the PE
                    # never waits head-of-line on the ACT evacuations.
                    for mi, dest in ((0, qt), (1, kt)):
                        tr1 = xtp.tile([128, c.NBLK], f32, tag="ropet1")
                        nc.vector.tensor_mul(
                            tr1[:], pss[mi][:], cosT[:, t0 + n0:t0 + n0 + c.NBLK])
                        sw = psx.tile([128, c.NBLK], f32, tag="aux")
                        nc.tensor.matmul(sw[:], pblk[:], raws[mi][:],
                                         start=True, stop=True)
                        tr2 = xtp.tile([128, c.NBLK], f32, tag="xsq")
                        nc.vector.tensor_mul(
                            tr2[:], sw[:], sinT[:, t0 + n0:t0 + n0 + c.NBLK])
                        nc.vector.tensor_add(dest[:, n0:n0 + c.NBLK], tr1[:], tr2[:])
                vts = []
                for tt in range(c.TKT_B):
                    vt = vtp.tile([128, 130], f16, tag="vtm")
                    ps = psmm.tile([128, 128], f32, tag="mm")
                    for k in range(c.KT):
                        nc.tensor.matmul(
                            ps[:],
                            h1[:, k * c.S + tt * 128:k * c.S + tt * 128 + 128],
                            wq[:, k * 3 * c.DSH + 256:k * 3 * c.DSH + 384],
                            start=(k == 0), stop=(k == c.KT - 1))
                    nc.vector.tensor_copy(vt[:, 0:64], ps[:, 0:64])
                    nc.vector.tensor_copy(vt[:, 65:129], ps[:, 64:128])
                    nc.vector.memset(vt[:, 64:65], 1.0)
                    nc.vector.memset(vt[:, 129:130], 1.0)
                    vts.append(vt)
                return qt, kt, vts

            def attn_stage(b, qt, kt, vts, lname):
                """Causal attention for this core's 2 heads; returns gathered
                o tile [128, KT*S] fp16 after AllGather."""
                oca = ocp.tile([64, c.S], f16, tag="oca")
                ocb = ocp.tile([64, c.S], f16, tag="ocb")
                for nb in range(c.NB_B):
                    n0 = nb * c.NBLK
                    ntk = (n0 + c.NBLK) // 128
                    ops_ = []
                    for _hi in range(2):
                        opt_ = psov.tile([65, c.NBLK], f32, tag="ops")
                        ops_.append(opt_)
                    prev = None
                    for j in range(ntk):
                        diag = j * 128 >= n0
                        cur = []
                        for hi in range(2):
                            h0 = hi * 64
                            sc = pssc.tile([128, c.NBLK], f32, tag="scps")
                            nc.tensor.matmul(
                                sc[:], kt[h0:h0 + 64, j * 128:j * 128 + 128],
                                qt[h0:h0 + 64, n0:n0 + c.NBLK],
                                start=True, stop=True)
                            if diag:
                                mo = (j * 128 - n0) // 128
                                nc.vector.tensor_add(
                                    sc[:], sc[:],
                                    masks[:, mo * c.NBLK:(mo + 1) * c.NBLK])
                            pr = probp.tile([128, c.NBLK], f16, tag="probs")
                            nc.scalar.activation(pr[:], sc[:], AF.Exp,
                                                 scale=float(1.0 / np.sqrt(DH)))
                            cur.append(pr)
                        if prev is not None:
                            jp, prs = prev
                            for hi in range(2):
                                nc.tensor.matmul(
                                    ops_[hi][:], vts[jp][:, hi * 65:hi * 65 + 65],
                                    prs[hi][:], start=(jp == 0), stop=False)
                        prev = (j, cur)
                    jp, prs = prev
                    for hi in range(2):
                        nc.tensor.matmul(
                            ops_[hi][:], vts[jp][:, hi * 65:hi * 65 + 65],
                            prs[hi][:], start=(jp == 0), stop=True)
                    for hi, oc in ((0, oca), (1, ocb)):
                        rr = rowsp.tile([1, c.NBLK], f16, tag="rr")
                        nc.vector.reciprocal(rr[:], ops_[hi][64:65, :])
                        rb = psx.tile([64, c.NBLK], f32, tag="aux")
                        nc.tensor.matmul(rb[:], ones_row[:, 0:64], rr[:],
                                         start=True, stop=True)
                        rbs = stagep.tile([64, c.NBLK], f16, tag="rbs")
                        nc.scalar.copy(rbs[:], rb[:])
                        nc.vector.tensor_mul(
                            oc[:, n0:n0 + c.NBLK], ops_[hi][0:64, :], rbs[:])
                ago_in = dram.tile([c.DSH, c.S], f16, tag="ago_in")
                ago_out = dram.tile([D, c.S], f16, tag="ago_out", addr_space=("Local" if c.no_coll else "Shared"))
                nc.sync.dma_start(ago_in[0:64, :], oca[:])
                nc.sync.dma_start(ago_in[64:128, :], ocb[:])
                _collective("AllGather", OP.bypass, ago_in, ago_out)
                of = gathp.tile([128, c.KT * c.S], f16, tag="gfull")
                _blocked_load(of, ago_out[:], c.KT, split=True)
                return of

            def proj_stage(b, of, wpr):
                """Column-sharded proj + in-place residual add."""
                t0 = b * c.S
                for nb in range(c.NB_B):
                    n0 = nb * c.NBLK
                    ps = psmm.tile([128, c.NBLK], f32, tag="mm")
                    for k in range(c.KT):
                        nc.tensor.matmul(
                            ps[:], wpr[:, k * c.DSH:k * c.DSH + 128],
                            of[:, k * c.S + n0:k * c.S + n0 + c.NBLK],
                            start=(k == 0), stop=(k == c.KT - 1))
                    nc.vector.tensor_add(
                        xres[:, t0 + n0:t0 + n0 + c.NBLK],
                        xres[:, t0 + n0:t0 + n0 + c.NBLK], ps[:])

            def fc1_defer_stage(b, xcf, arow, brow, w1, cw1):
                ga = gactp.tile([128, c.F1T * c.S], f16, tag="gact")
                for nb in range(c.NB_B):
                    n0 = nb * c.NBLK
                    abps = psx.tile([128, c.NBLK], f32, tag="aux")
                    nc.tensor.matmul(abps[:], ones_row[:],
                                     arow[0:1, n0:n0 + c.NBLK], start=True, stop=True)
                    absb = stagep.tile([128, c.NBLK], f16, tag="absb")
                    nc.scalar.copy(absb[:], abps[:])
                    for m in range(c.F1T):
                        ps = psmm.tile([128, c.NBLK], f32, tag="mm")
                        for k in range(c.KT):
                            nc.tensor.matmul(
                                ps[:],
                                w1[:, k * c.F1 + m * 128:k * c.F1 + m * 128 + 128],
                                xcf[:, k * c.S + n0:k * c.S + n0 + c.NBLK],
                                start=(k == 0), stop=False)
                        nc.tensor.matmul(
                            ps[:], cw1[0:1, m * 128:m * 128 + 128],
                            brow[0:1, n0:n0 + c.NBLK], start=False, stop=True)
                        tmp = xtp.tile([128, c.NBLK], f32, tag="ropet1")
                        nc.vector.tensor_mul(tmp[:], ps[:], absb[:])
                        nc.scalar.activation(
                            ga[:, m * c.S + n0:m * c.S + n0 + c.NBLK],
                            tmp[:], AF.Gelu)
                return ga

            def fc1_stage(b, h2, w1):
                ga = gactp.tile([128, c.F1T * c.S], f16, tag="gact")
                for nb in range(c.NB_B):
                    n0 = nb * c.NBLK
                    for m in range(c.F1T):
                        ps = psmm.tile([128, c.NBLK], f32, tag="mm")
                        for k in range(c.KT):
                            nc.tensor.matmul(
                                ps[:],
                                w1[:, k * c.F1 + m * 128:k * c.F1 + m * 128 + 128],
                                h2[:, k * c.S + n0:k * c.S + n0 + c.NBLK],
                                start=(k == 0), stop=(k == c.KT - 1))
                        nc.scalar.activation(
                            ga[:, m * c.S + n0:m * c.S + n0 + c.NBLK],
                            ps[:], AF.Gelu)
                return ga

            def fc2_stage(b, ga, w2):
                """Partial fc2 -> ReduceScatter; returns rs_out DRAM tile."""
                rs_in = dram.tile([D, c.S], f16, tag="rs_in")
                rs_out = dram.tile([c.DSH, c.S], f16, tag="rs_out")
                for nb in range(c.NB_B):
                    n0 = nb * c.NBLK
                    for m in range(c.DT):
                        ps = psmm.tile([128, c.NBLK], f32, tag="mm")
                        for k in range(c.F1T):
                            nc.tensor.matmul(
                                ps[:], w2[:, k * D + m * 128:k * D + m * 128 + 128],
                                ga[:, k * c.S + n0:k * c.S + n0 + c.NBLK],
                                start=(k == 0), stop=(k == c.F1T - 1))
                        mp = stagep.tile([128, c.NBLK], f16, tag="mlpstg")
                        nc.vector.tensor_copy(mp[:], ps[:])
                        nc.sync.dma_start(
                            rs_in[m * 128:m * 128 + 128, n0:n0 + c.NBLK], mp[:])
                _collective("ReduceScatter", OP.add, rs_in, rs_out)
                return rs_out

            def xupdate_stage(b, rs_out):
                t0 = b * c.S
                rsb = xtp.tile([c.DSH, c.S], f16, tag="rsb")
                nc.sync.dma_start(rsb[:], rs_out[:])
                nc.vector.tensor_add(
                    xres[:, t0:t0 + c.S], xres[:, t0:t0 + c.S], rsb[:])

            # ================= layer loop (batch-pipelined) =================
            for l in range(c.L):
                wq = wqp.tile([128, c.KT * 3 * c.DSH], f16, tag="wq")
                _blocked_load(wq, wqkvT_d[l], c.KT)
                wpr = wpp.tile([128, c.KT * c.DSH], f16, tag="wp")
                _blocked_load(wpr, wprojT_d[l], c.KT)
                w1 = w1p.tile([128, c.KT * c.F1], f16, tag="w1")
                _blocked_load(w1, wfc1T_d[l], c.KT)
                cw1 = w1p.tile([1, c.F1], f16, tag="cw1")
                nc.sync.dma_start(cw1[:], cw1_d[l:l + 1, :])
                w2 = w2p.tile([128, c.F1T * D], f16, tag="w2")
                _blocked_load(w2, wfc2T_d[l], c.F1T)

                st = {b: ln_stats(b)[0] for b in range(B)}
                h1 = {b: ln_apply(b, st[b], "h1") for b in range(B)}
                if "attn" not in c.skip:
                    qkv = {b: qkv_stage(b, h1[b], wq) for b in range(B)}
                    of = {b: attn_stage(b, *qkv[b], "o") for b in range(B)}
                    for b in range(B):
                        proj_stage(b, of[b], wpr)
                st2 = {b: ln_stats(b, defer=True) for b in range(B)}
                ab2 = {b: ln_apply(b, st2[b][0], "h2", defer=True) for b in range(B)}
                if "mlp" not in c.skip:
                    ga = {b: fc1_defer_stage(b, st2[b][1], ab2[b][0],
                                             ab2[b][1], w1, cw1)
                          for b in range(B)}
                    rs = {b: fc2_stage(b, ga[b], w2) for b in range(B)}
                    for b in range(B):
                        xupdate_stage(b, rs[b])

            # ================= final LN + LM head =================
            stf = {b: ln_stats(b, defer=True) for b in range(B)}
            abf = {b: ln_apply(b, stf[b][0], "hf", defer=True) for b in range(B)}
            nchunk_f = c.S // 128
            acolT = {}
            for b in range(B):
                scr = dram.tile([nchunk_f, 128], f32, tag="acolscr")
                nc.sync.dma_start(scr[:], abf[b][2][:])
                act = consts.tile([128, nchunk_f], f32, tag=f"acolT{b}")
                nc.sync.dma_start(act[:], scr[:].rearrange("j p -> p j"))
                acolT[b] = act
            run_head = "head" not in c.skip
            HNB = max(c.NBLK, 512) if c.S >= 512 else c.NBLK   # head vocab block
            nvb = (c.VS + HNB - 1) // HNB if run_head else 0
            for b in range(B if run_head else 0):
                xcf, brow = stf[b][1], abf[b][1]
                for vb in range(nvb):
                    v0 = vb * HNB
                    vn = min(HNB, c.VS - v0)
                    er = embp.tile([128, c.KT * HNB], f16, tag="emb")
                    nc.sync.dma_start(
                        er[:, 0:c.KT * vn].rearrange("p (k n) -> p k n", k=c.KT),
                        embT_d[:, v0:v0 + vn].rearrange("(k p) n -> p k n", p=128))
                    cer = rowsp.tile([1, HNB], f16, tag="cerow")
                    nc.sync.dma_start(cer[:, 0:vn], cemb_d[:, v0:v0 + vn])
                    for tt in range(c.S // 128):
                        ps = psmm.tile([128, HNB], f32, tag="mm")
                        for k in range(c.KT):
                            nc.tensor.matmul(
                                ps[:, 0:vn],
                                xcf[:, k * c.S + tt * 128:k * c.S + tt * 128 + 128],
                                er[:, k * vn:k * vn + vn],
                                start=(k == 0), stop=False)
                        nc.tensor.matmul(
                            ps[:, 0:vn], brow[0:1, tt * 128:tt * 128 + 128],
                            cer[:, 0:vn], start=False, stop=True)
                        lg = stagep.tile([128, HNB], f32, tag="lgstg")
                        nc.vector.tensor_scalar_mul(
                            lg[:, 0:vn], ps[:, 0:vn],
                            acolT[b][:, tt:tt + 1])
                        nc.sync.dma_start(
                            out_d[b * c.S + tt * 128:b * c.S + tt * 128 + 128,
                                  v0:v0 + vn],
                            lg[:, 0:vn])

    nc.compile()
    return nc


# ======================= host side =======================

def _rope_host(cfg: Cfg):
    """cosT/sinT [128, T] for the per-core q/k layout (2 heads stacked),
    rope rotation sign folded into sinT."""
    c = cfg
    theta = 10000.0
    inv_freq = 1.0 / theta ** (np.arange(0, DH, 2, dtype=np.float32) / DH)
    t = np.arange(c.S, dtype=np.float32)
    freqs = np.outer(t, inv_freq)                      # [S, 32]
    emb = np.concatenate([freqs, freqs], axis=-1)      # [S, 64]
    cos = np.cos(emb).T.astype(np.float32)             # [64, S]
    sin = np.sin(emb).T.astype(np.float32)
    sinp = sin.copy()
    sinp[:DH // 2] = -sin[:DH // 2]                    # row dh<32 gets -sin
    cosT = np.tile(cos, (2, B)).astype(np.float16)     # [128, T]
    sinT = np.tile(sinp, (2, B)).astype(np.float16)
    return cosT, sinT


def _masks_host(cfg: Cfg):
    c = cfg
    nmask = c.NBLK // 128
    m = np.zeros((128, nmask, c.NBLK), dtype=np.float32)
    for d in range(nmask):
        tk = d * 128 + np.arange(128)[:, None]
        tq = np.arange(c.NBLK)[None, :]
        m[:, d, :] = np.where(tk <= tq, 0.0, MASK_NEG)
    return m.astype(np.float16)


def _pblk_host():
    p64 = np.zeros((64, 64), dtype=np.float32)
    p64[:32, 32:] = np.eye(32)
    p64[32:, :32] = np.eye(32)
    # lhsT convention: out = lhsT.T @ x ; we want out = P @ x -> lhsT = P.T
    pb = np.zeros((128, 128), dtype=np.float32)
    pb[:64, :64] = p64.T
    pb[64:, 64:] = p64.T
    return pb.astype(np.float16)


def prep_in_maps(cfg: Cfg, inputs):
    """Shard/cast/transpose the full inputs into 8 per-core in_maps."""
    c = cfg
    ids = np.asarray(inputs["input_ids"]).astype(np.int64)
    emb = np.asarray(inputs["tok_emb"], dtype=np.float32)
    Wqkv = np.asarray(inputs["Wqkv"], dtype=np.float32)
    Wproj = np.asarray(inputs["Wproj"], dtype=np.float32)
    Wfc1 = np.asarray(inputs["Wfc1"], dtype=np.float32)
    Wfc2 = np.asarray(inputs["Wfc2"], dtype=np.float32)
    ln1_g = np.asarray(inputs["ln1_g"], dtype=np.float32)
    ln2_g = np.asarray(inputs["ln2_g"], dtype=np.float32)
    lnf_g = np.asarray(inputs["lnf_g"], dtype=np.float32)
    for nm in ("ln1_b", "ln2_b", "lnf_b"):
        assert np.abs(np.asarray(inputs[nm])).max() == 0.0, "LN biases must be 0"

    x0 = emb[ids.reshape(-1)]                          # [T, D]
    x0T = np.ascontiguousarray(x0.T)                   # [D, T]

    cosT, sinT = _rope_host(c)
    masksH = _masks_host(c)
    pblk = _pblk_host()

    # fold LN gains into following weights
    Wqkv_f = Wqkv * ln1_g[:, None, :]                  # [L, 3D, D]
    Wfc1_f = Wfc1 * ln2_g[:, None, :]
    emb_f = emb * lnf_g[None, :]

    in_maps = []
    for cc in range(NC):
        hsl = slice(cc * c.DSH, (cc + 1) * c.DSH)      # head-dim / feature slice
        q = Wqkv_f[:, hsl, :]                          # [L, 128, D]
        k = Wqkv_f[:, D + cc * c.DSH:D + (cc + 1) * c.DSH, :]
        v = Wqkv_f[:, 2 * D + cc * c.DSH:2 * D + (cc + 1) * c.DSH, :]
        wqkvT = np.concatenate([q, k, v], axis=1).transpose(0, 2, 1)  # [L, D, 384]
        wprojT = Wproj[:, hsl, :].transpose(0, 2, 1)   # [L, D, 128]
        wfc1T = Wfc1_f[:, cc * c.F1:(cc + 1) * c.F1, :].transpose(0, 2, 1)
        cw1 = Wfc1_f[:, cc * c.F1:(cc + 1) * c.F1, :].sum(axis=2)  # [L, F1]
        wfc2T = Wfc2[:, :, cc * c.F1:(cc + 1) * c.F1].transpose(0, 2, 1)  # [L, F1, D]
        embT = emb_f[cc * c.VS:(cc + 1) * c.VS, :].T   # [D, VS]
        cemb = emb_f[cc * c.VS:(cc + 1) * c.VS, :].sum(axis=1)[None, :]  # [1, VS]
        in_maps.append({
            "x0": np.ascontiguousarray(x0T[hsl]).astype(np.float32),
            "wqkvT": np.ascontiguousarray(wqkvT).astype(np.float16),
            "wprojT": np.ascontiguousarray(wprojT).astype(np.float16),
            "wfc1T": np.ascontiguousarray(wfc1T).astype(np.float16),
            "cw1": np.ascontiguousarray(cw1).astype(np.float16),
            "wfc2T": np.ascontiguousarray(wfc2T).astype(np.float16),
            "embT": np.ascontiguousarray(embT).astype(np.float16),
            "cemb": np.ascontiguousarray(cemb).astype(np.float16),
            "cosT": cosT, "sinT": sinT,
            "masks": masksH, "pblk": pblk,
        })
    return in_maps


_PROG_CACHE = {}


def _get_program(cfg: Cfg):
    key = (cfg.V, cfg.L, cfg.S, cfg.NBLK, cfg.no_coll, tuple(sorted(cfg.skip)))
    if key not in _PROG_CACHE:
        _PROG_CACHE[key] = build_program(cfg)
    return _PROG_CACHE[key]


def run(cfg: Cfg, inputs, trace=False):
    nc = _get_program(cfg)
    in_maps = prep_in_maps(cfg, inputs)
    res = run_bass_kernel_spmd(nc, in_maps, core_ids=list(range(NC)), trace=trace)
    parts = [res.results[cc]["logits"] for cc in range(NC)]
    logits = np.concatenate(parts, axis=1)             # [T, V]
    return logits.reshape(B, cfg.S, cfg.V), res


def kernel(**inputs) -> np.ndarray:
    cfg = Cfg()
    logits, _ = run(cfg, inputs)
    return logits
